# revision 14
# baseline (speedup 1.0000x reference)
"""Trainium2 Bass kernel for CornerBoundingBoxEMDLoss.

For each sample: 8x8 pairwise corner distances, then exact min-cost perfect
matching via meet-in-the-middle:

  min over perms = min over 70 4-subsets T of
      (min assignment of preds {0,1,2,3} onto T)
    + (min assignment of preds {4,5,6,7} onto complement(T))

computed hierarchically: pred pairs -> target pairs (L1, one-hot GEMM with
two orderings + elementwise min), pairs -> quads (L2, one-hot GEMM over the
6 = C(4,2) pair-to-half assignments per 2+2 split + group-min), then a fused
add+min reduction over the 70 complement-aligned A+B sums (L3). Exact same
minimum as brute force over 8! permutations, ~50x less arithmetic.

Data-parallel across 8 NeuronCores: 512 samples per core, processed as
4 chunks of 128 samples (samples on SBUF partitions; d2 rows of two chunks
are transposed together in one PE pass to coord-major for the selection
GEMMs). Selection GEMMs run in bf16 (one-hot weights are exact in bf16;
distances round to ~0.4% which is well inside the 2e-2 gate). All inputs
arrive in 3 packed DMAs; the output leaves as one [128,4] DMA that the host
reorders.
"""

import itertools

import numpy as np
import ml_dtypes

import concourse.bacc as bacc
import concourse.mybir as mybir
import concourse.tile as tile

N_CORES = 8
B_TOTAL = 4096
B_CORE = B_TOTAL // N_CORES          # 512
N_CHUNKS = 4
CHUNK = B_CORE // N_CHUNKS           # 128

F32 = mybir.dt.float32
BF16 = mybir.dt.bfloat16

MIN_INIT = 1.0e30


def _build_constants():
    """Packed one-hot selection matrices + identity.

    cpack [128, 1064] bf16:
      cols   0:112  l1 ordering 0   (partitions 0:64 and replicated 64:128)
      cols 112:224  l1 ordering 1   (same replication)
      cols 224:1064 l2 (partitions 0:112): 840 = [A-side 70*6 | B-side 70*6]
    ident [128, 128] f32 for PE transposes.
    """
    pairs = list(itertools.combinations(range(8), 2))            # 28
    pair_idx = {p: i for i, p in enumerate(pairs)}
    subs4 = list(itertools.combinations(range(8), 4))            # 70
    pred_pairs = [(0, 1), (2, 3), (4, 5), (6, 7)]

    l1o0 = np.zeros((64, 112), dtype=np.float32)
    l1o1 = np.zeros((64, 112), dtype=np.float32)
    for q, (i0, i1) in enumerate(pred_pairs):
        for p, (a, b) in enumerate(pairs):
            col = q * 28 + p
            l1o0[i0 * 8 + a, col] = 1; l1o0[i1 * 8 + b, col] = 1
            l1o1[i0 * 8 + b, col] = 1; l1o1[i1 * 8 + a, col] = 1

    # all 6 C(T,2) choices of which target pair the first pred pair gets
    # (each 2+2 split appears twice with the pair roles swapped -- those are
    # distinct matchings, both needed)
    l2 = np.zeros((112, 840), dtype=np.float32)
    for t, T in enumerate(subs4):
        for s, S in enumerate(itertools.combinations(T, 2)):
            R = tuple(sorted(set(T) - set(S)))
            l2[0 * 28 + pair_idx[S], t * 6 + s] = 1
            l2[1 * 28 + pair_idx[R], t * 6 + s] = 1
        TB = tuple(sorted(set(range(8)) - set(T)))               # complement
        for s, S in enumerate(itertools.combinations(TB, 2)):
            R = tuple(sorted(set(TB) - set(S)))
            l2[2 * 28 + pair_idx[S], 420 + t * 6 + s] = 1
            l2[3 * 28 + pair_idx[R], 420 + t * 6 + s] = 1

    cpack = np.zeros((128, 1064), dtype=np.float32)
    cpack[0:64, 0:112] = l1o0
    cpack[0:64, 112:224] = l1o1
    cpack[64:128, 0:224] = cpack[0:64, 0:224]
    cpack[0:112, 224:1064] = l2
    cpack = cpack.astype(ml_dtypes.bfloat16)

    ident = np.eye(128, dtype=np.float32)
    return cpack, ident


def build_nc():
    nc = bacc.Bacc("TRN2", target_bir_lowering=False, debug=False)

    # data: per chunk c the 48-col block [pred_c | -targ_c], see kernel()
    data_d = nc.dram_tensor("data", [CHUNK, 48 * N_CHUNKS], F32,
                            kind="ExternalInput")
    cpack_d = nc.dram_tensor("cpack", [128, 1064], BF16, kind="ExternalInput")
    id_d = nc.dram_tensor("ident", [128, 128], F32, kind="ExternalInput")
    out_d = nc.dram_tensor("out", [CHUNK, N_CHUNKS], F32, kind="ExternalOutput")

    with tile.TileContext(nc) as tc:
        with (
            tc.tile_pool(name="consts", bufs=1) as cpool,
            tc.tile_pool(name="persist", bufs=1) as ppool,
            tc.tile_pool(name="work", bufs=3) as wpool,
            tc.tile_pool(name="pairs", bufs=2) as qpool,
            tc.tile_pool(name="psum_t", bufs=1, space="PSUM") as pst,
            tc.tile_pool(name="psum_a", bufs=2, space="PSUM") as psa,
            tc.tile_pool(name="psum_l2", bufs=2, space="PSUM") as psl2,
        ):
            data = cpool.tile([CHUNK, 192], F32, tag="data")
            cpk = cpool.tile([128, 1064], BF16, tag="cpack")
            c_id = cpool.tile([128, 128], F32, tag="ident")
            # data first (gates the whole pipeline), on the sync queue;
            # consts on the scalar queue in parallel; ident second on sync.
            nc.sync.dma_start(data[:, 0:48], data_d[:, 0:48])
            nc.sync.dma_start(data[:, 48:96], data_d[:, 48:96])
            nc.sync.dma_start(data[:, 96:192], data_d[:, 96:192])
            nc.sync.dma_start(c_id[:, :], id_d[:, :])
            nc.sync.dma_start(cpk[:, :], cpack_d[:, :])

            m_t = ppool.tile([112, B_CORE], BF16, tag="m")
            loss = ppool.tile([128, N_CHUNKS], F32, tag="loss")
            tiny = ppool.tile([1, 1], F32, tag="tiny")

            # Force the (single) act table that holds sqrt+square+copy: the
            # table-load pass picks the table of the first activation, and
            # sqrt_and_others covers everything we use.
            nc.gpsimd.memset(tiny[:, :], 1.0)
            nc.scalar.activation(tiny[:, :], tiny[:, :],
                                 mybir.ActivationFunctionType.Sqrt)

            d2p = [None, None]
            dtp = [None, None]

            def phase1(c):
                """distances^2 for chunk c -> d2 pair tile column half."""
                pair, half = divmod(c, 2)
                if half == 0:
                    d2p[pair] = qpool.tile([CHUNK, 128], F32, tag="d2p", name="d2p")
                dsl = data[:, 48 * c: 48 * c + 48]
                diff = wpool.tile([CHUNK, 192], F32, tag="diff")
                p_b = (dsl[:, 0:24].rearrange("p (i c) -> p i c", i=8)
                       .unsqueeze(2).broadcast_to((CHUNK, 8, 8, 3)))
                t_b = (dsl[:, 24:48].rearrange("p (j c) -> p j c", j=8)
                       .unsqueeze(1).broadcast_to((CHUNK, 8, 8, 3)))
                d4 = diff[:, :].rearrange("p (i j c) -> p i j c", i=8, j=8)
                nc.gpsimd.tensor_add(d4, p_b, t_b)

                sq = wpool.tile([CHUNK, 192], BF16, tag="sq")
                nc.scalar.activation(sq[:, :], diff[:, :],
                                     mybir.ActivationFunctionType.Square)
                nc.vector.tensor_reduce(
                    d2p[pair][:, 64 * half: 64 * half + 64],
                    sq[:, :].rearrange("p (r c) -> p r c", c=3),
                    axis=mybir.AxisListType.X, op=mybir.AluOpType.add)

            def transpose_pair(pair):
                """[128 samples, 2x64 d2] -> bf16 dist [2x64, 128 samples]."""
                tp = pst.tile([128, 128], F32, tag="tp")
                nc.tensor.transpose(tp[:, :], d2p[pair][:, :], c_id[:, :])
                dtp[pair] = qpool.tile([128, 128], BF16, tag="dtp", name="dtp")
                nc.scalar.activation(dtp[pair][:, :], tp[:, :],
                                     mybir.ActivationFunctionType.Sqrt)

            def l1(c):
                """pred-pair x target-pair costs for chunk c -> m_t cols."""
                pair, half = divmod(c, 2)
                hp = slice(64 * half, 64 * half + 64)
                rhs = dtp[pair][hp, :]
                ps01 = psa.tile([112, 256], F32, tag="ps01")
                nc.tensor.matmul(ps01[:, 0:128], cpk[hp, 0:112], rhs,
                                 start=True, stop=True)
                nc.tensor.matmul(ps01[:, 128:256], cpk[hp, 112:224], rhs,
                                 start=True, stop=True)
                # HW: TensorTensor may read at most one input from PSUM
                s1 = wpool.tile([112, 128], F32, tag="s1")
                nc.scalar.activation(s1[:, :], ps01[:, 128:256],
                                     mybir.ActivationFunctionType.Copy)
                nc.vector.tensor_tensor(
                    m_t[:, CHUNK * c: CHUNK * (c + 1)], ps01[:, 0:128],
                    s1[:, :], op=mybir.AluOpType.min)

            def l2l3(c):
                """quad costs + final min for chunk c."""
                ps2 = psl2.tile([128, 1024], F32, tag="ps2")
                msl = m_t[:, CHUNK * c: CHUNK * (c + 1)]
                nc.tensor.matmul(ps2[:, 0:420], msl, cpk[0:112, 224:644],
                                 start=True, stop=True)
                nc.tensor.matmul(ps2[:, 512:932], msl, cpk[0:112, 644:1064],
                                 start=True, stop=True)
                minab = wpool.tile([128, 140], BF16, tag="minab")
                v = (ps2[:, :].rearrange("p (h x) -> p h x", h=2)[:, :, 0:420]
                     .rearrange("p h (t s) -> p h t s", s=6))
                nc.vector.tensor_reduce(
                    minab[:, :], v,
                    axis=mybir.AxisListType.X, op=mybir.AluOpType.min)
                scratch = wpool.tile([128, 70], BF16, tag="scratch")
                nc.vector.tensor_tensor(scratch[:, :], minab[:, 0:70],
                                        minab[:, 70:140],
                                        op=mybir.AluOpType.add)
                nc.vector.tensor_reduce(loss[:, c:c + 1], scratch[:, :],
                                        axis=mybir.AxisListType.X,
                                        op=mybir.AluOpType.min)

            # pipelined schedule (engine streams stay dependency-ordered):
            phase1(0); phase1(1)
            transpose_pair(0)
            phase1(2); phase1(3)
            l1(0); l1(1)
            transpose_pair(1)
            l2l3(0); l2l3(1)
            l1(2); l1(3)
            l2l3(2); l2l3(3)

            # loss[p, c] -> dram [p, c]; host reorders to c*128+p
            nc.sync.dma_start(out_d[:, :], loss[:, :])

    nc.compile()
    return nc


_NC = None


def _get_nc():
    global _NC
    if _NC is None:
        _NC = build_nc()
    return _NC


def _input_maps(pred_corners, target_corners):
    cpack, ident = _build_constants()
    pred = np.ascontiguousarray(pred_corners, dtype=np.float32)
    targ = np.ascontiguousarray(target_corners, dtype=np.float32)
    in_maps = []
    for k in range(N_CORES):
        sl = slice(k * B_CORE, (k + 1) * B_CORE)
        # [4 chunks, 128 slots, 24] -> [128, 4*48] with per-chunk blocks
        # [pred_c | -targ_c]
        pk = pred[sl].reshape(N_CHUNKS, CHUNK, 24)
        tk = targ[sl].reshape(N_CHUNKS, CHUNK, 24)
        datak = np.concatenate([pk, -tk], axis=2)          # [4, 128, 48]
        datak = np.ascontiguousarray(
            datak.transpose(1, 0, 2).reshape(CHUNK, 192))
        in_maps.append({"data": datak, "cpack": cpack, "ident": ident})
    return in_maps


def _gather(results):
    outs = []
    for k in range(N_CORES):
        o = results[k]["out"].reshape(CHUNK, N_CHUNKS)
        outs.append(np.ascontiguousarray(o.T).reshape(B_CORE))
    return np.concatenate(outs)


def kernel(pred_corners: np.ndarray, target_corners: np.ndarray) -> np.ndarray:
    from concourse.bass_utils import run_bass_kernel_spmd

    nc = _get_nc()
    in_maps = _input_maps(pred_corners, target_corners)
    res = run_bass_kernel_spmd(nc, in_maps, core_ids=list(range(N_CORES)))
    return _gather(res.results)


# revision 15
# speedup vs baseline: 1.0364x; 1.0364x over previous
"""Trainium2 Bass kernel for CornerBoundingBoxEMDLoss.

For each sample: 8x8 pairwise corner distances, then exact min-cost perfect
matching via meet-in-the-middle:

  min over perms = min over 70 4-subsets T of
      (min assignment of preds {0,1,2,3} onto T)
    + (min assignment of preds {4,5,6,7} onto complement(T))

computed hierarchically: pred pairs -> target pairs (L1, one-hot GEMM with
two orderings + elementwise min), pairs -> quads (L2, one-hot GEMM over the
6 = C(4,2) pair-to-half assignments per 2+2 split + group-min), then a fused
add+min reduction over the 70 complement-aligned A+B sums (L3). Exact same
minimum as brute force over 8! permutations, ~50x less arithmetic.

Data-parallel across 8 NeuronCores: 512 samples per core, processed as
4 chunks of 128 samples (samples on SBUF partitions; d2 rows of two chunks
are transposed together in one PE pass to coord-major for the selection
GEMMs). Selection GEMMs run in bf16 (one-hot weights are exact in bf16;
distances round to ~0.4% which is well inside the 2e-2 gate). All inputs
arrive in 3 packed DMAs; the output leaves as one [128,4] DMA that the host
reorders.
"""

import itertools

import numpy as np
import ml_dtypes

import concourse.bacc as bacc
import concourse.mybir as mybir
import concourse.tile as tile

N_CORES = 8
B_TOTAL = 4096
B_CORE = B_TOTAL // N_CORES          # 512
N_CHUNKS = 4
CHUNK = B_CORE // N_CHUNKS           # 128

F32 = mybir.dt.float32
BF16 = mybir.dt.bfloat16

MIN_INIT = 1.0e30


def _build_constants():
    """Packed one-hot selection matrices + identity.

    cpack [128, 1064] bf16:
      cols   0:112  l1 ordering 0   (partitions 0:64 and replicated 64:128)
      cols 112:224  l1 ordering 1   (same replication)
      cols 224:1064 l2 (partitions 0:112): 840 = [A-side 70*6 | B-side 70*6]
    ident [128, 128] f32 for PE transposes.
    """
    pairs = list(itertools.combinations(range(8), 2))            # 28
    pair_idx = {p: i for i, p in enumerate(pairs)}
    subs4 = list(itertools.combinations(range(8), 4))            # 70
    pred_pairs = [(0, 1), (2, 3), (4, 5), (6, 7)]

    l1o0 = np.zeros((64, 112), dtype=np.float32)
    l1o1 = np.zeros((64, 112), dtype=np.float32)
    for q, (i0, i1) in enumerate(pred_pairs):
        for p, (a, b) in enumerate(pairs):
            col = q * 28 + p
            l1o0[i0 * 8 + a, col] = 1; l1o0[i1 * 8 + b, col] = 1
            l1o1[i0 * 8 + b, col] = 1; l1o1[i1 * 8 + a, col] = 1

    # all 6 C(T,2) choices of which target pair the first pred pair gets
    # (each 2+2 split appears twice with the pair roles swapped -- those are
    # distinct matchings, both needed)
    l2 = np.zeros((112, 840), dtype=np.float32)
    for t, T in enumerate(subs4):
        for s, S in enumerate(itertools.combinations(T, 2)):
            R = tuple(sorted(set(T) - set(S)))
            l2[0 * 28 + pair_idx[S], t * 6 + s] = 1
            l2[1 * 28 + pair_idx[R], t * 6 + s] = 1
        TB = tuple(sorted(set(range(8)) - set(T)))               # complement
        for s, S in enumerate(itertools.combinations(TB, 2)):
            R = tuple(sorted(set(TB) - set(S)))
            l2[2 * 28 + pair_idx[S], 420 + t * 6 + s] = 1
            l2[3 * 28 + pair_idx[R], 420 + t * 6 + s] = 1

    cpack = np.zeros((128, 1064), dtype=np.float32)
    cpack[0:64, 0:112] = l1o0
    cpack[0:64, 112:224] = l1o1
    cpack[64:128, 0:224] = cpack[0:64, 0:224]
    cpack[0:112, 224:1064] = l2
    cpack = cpack.astype(ml_dtypes.bfloat16)

    ident = np.eye(128, dtype=np.float32)
    return cpack, ident


def build_nc():
    nc = bacc.Bacc("TRN2", target_bir_lowering=False, debug=False)

    # data: per chunk c the 48-col block [pred_c | -targ_c], see kernel()
    data_d = nc.dram_tensor("data", [CHUNK, 48 * N_CHUNKS], F32,
                            kind="ExternalInput")
    cpack_d = nc.dram_tensor("cpack", [128, 1064], BF16, kind="ExternalInput")
    id_d = nc.dram_tensor("ident", [128, 128], F32, kind="ExternalInput")
    out_d = nc.dram_tensor("out", [CHUNK, N_CHUNKS], F32, kind="ExternalOutput")

    with tile.TileContext(nc) as tc:
        with (
            tc.tile_pool(name="consts", bufs=1) as cpool,
            tc.tile_pool(name="persist", bufs=1) as ppool,
            tc.tile_pool(name="work", bufs=3) as wpool,
            tc.tile_pool(name="pairs", bufs=2) as qpool,
            tc.tile_pool(name="psum_t", bufs=1, space="PSUM") as pst,
            tc.tile_pool(name="psum_a", bufs=2, space="PSUM") as psa,
            tc.tile_pool(name="psum_l2", bufs=2, space="PSUM") as psl2,
        ):
            data = cpool.tile([CHUNK, 192], F32, tag="data")
            cpk = cpool.tile([128, 1064], BF16, tag="cpack")
            c_id = cpool.tile([128, 128], F32, tag="ident")
            # data first (gates the whole pipeline), on the sync queue;
            # consts on the scalar queue in parallel; ident second on sync.
            nc.sync.dma_start(data[:, :], data_d[:, :])
            nc.scalar.dma_start(cpk[:, :], cpack_d[:, :])
            nc.sync.dma_start(c_id[:, :], id_d[:, :])

            m_t = ppool.tile([112, B_CORE], BF16, tag="m")
            loss = ppool.tile([128, N_CHUNKS], F32, tag="loss")
            tiny = ppool.tile([1, 1], F32, tag="tiny")

            # Force the (single) act table that holds sqrt+square+copy: the
            # table-load pass picks the table of the first activation, and
            # sqrt_and_others covers everything we use.
            nc.gpsimd.memset(tiny[:, :], 1.0)
            nc.scalar.activation(tiny[:, :], tiny[:, :],
                                 mybir.ActivationFunctionType.Sqrt)

            d2p = [None, None]
            dtp = [None, None]

            def phase1(pair):
                """distances^2 for chunks 2p,2p+1 -> d2 pair tile."""
                diff = wpool.tile([CHUNK, 384], F32, tag="diff")
                for half in (0, 1):
                    c = 2 * pair + half
                    dsl = data[:, 48 * c: 48 * c + 48]
                    p_b = (dsl[:, 0:24].rearrange("p (i c) -> p i c", i=8)
                           .unsqueeze(2).broadcast_to((CHUNK, 8, 8, 3)))
                    t_b = (dsl[:, 24:48].rearrange("p (j c) -> p j c", j=8)
                           .unsqueeze(1).broadcast_to((CHUNK, 8, 8, 3)))
                    d4 = (diff[:, 192 * half: 192 * half + 192]
                          .rearrange("p (i j c) -> p i j c", i=8, j=8))
                    nc.gpsimd.tensor_add(d4, p_b, t_b)

                sq = wpool.tile([CHUNK, 384], BF16, tag="sq")
                nc.scalar.activation(sq[:, :], diff[:, :],
                                     mybir.ActivationFunctionType.Square)
                d2p[pair] = qpool.tile([CHUNK, 128], F32, tag="d2p",
                                       name="d2p")
                nc.vector.tensor_reduce(
                    d2p[pair][:, :].rearrange("p (h r) -> p h r", h=2),
                    sq[:, :].rearrange("p (h r c) -> p h r c", h=2, c=3),
                    axis=mybir.AxisListType.X, op=mybir.AluOpType.add)

            def transpose_pair(pair):
                """[128 samples, 2x64 d2] -> bf16 dist [2x64, 128 samples]."""
                tp = pst.tile([128, 128], F32, tag="tp")
                nc.tensor.transpose(tp[:, :], d2p[pair][:, :], c_id[:, :])
                dtp[pair] = qpool.tile([128, 128], BF16, tag="dtp", name="dtp")
                nc.scalar.activation(dtp[pair][:, :], tp[:, :],
                                     mybir.ActivationFunctionType.Sqrt)

            def l1(c):
                """pred-pair x target-pair costs for chunk c -> m_t cols."""
                pair, half = divmod(c, 2)
                hp = slice(64 * half, 64 * half + 64)
                rhs = dtp[pair][hp, :]
                ps01 = psa.tile([112, 256], F32, tag="ps01")
                nc.tensor.matmul(ps01[:, 0:128], cpk[hp, 0:112], rhs,
                                 start=True, stop=True)
                nc.tensor.matmul(ps01[:, 128:256], cpk[hp, 112:224], rhs,
                                 start=True, stop=True)
                # HW: TensorTensor may read at most one input from PSUM
                s1 = wpool.tile([112, 128], F32, tag="s1")
                nc.scalar.activation(s1[:, :], ps01[:, 128:256],
                                     mybir.ActivationFunctionType.Copy)
                nc.vector.tensor_tensor(
                    m_t[:, CHUNK * c: CHUNK * (c + 1)], ps01[:, 0:128],
                    s1[:, :], op=mybir.AluOpType.min)

            def l2l3(c):
                """quad costs + final min for chunk c."""
                ps2 = psl2.tile([128, 1024], F32, tag="ps2")
                msl = m_t[:, CHUNK * c: CHUNK * (c + 1)]
                nc.tensor.matmul(ps2[:, 0:420], msl, cpk[0:112, 224:644],
                                 start=True, stop=True)
                nc.tensor.matmul(ps2[:, 512:932], msl, cpk[0:112, 644:1064],
                                 start=True, stop=True)
                minab = wpool.tile([128, 140], BF16, tag="minab")
                v = (ps2[:, :].rearrange("p (h x) -> p h x", h=2)[:, :, 0:420]
                     .rearrange("p h (t s) -> p h t s", s=6))
                nc.vector.tensor_reduce(
                    minab[:, :], v,
                    axis=mybir.AxisListType.X, op=mybir.AluOpType.min)
                scratch = wpool.tile([128, 70], BF16, tag="scratch")
                nc.vector.tensor_tensor(scratch[:, :], minab[:, 0:70],
                                        minab[:, 70:140],
                                        op=mybir.AluOpType.add)
                nc.vector.tensor_reduce(loss[:, c:c + 1], scratch[:, :],
                                        axis=mybir.AxisListType.X,
                                        op=mybir.AluOpType.min)

            # pipelined schedule (engine streams stay dependency-ordered):
            phase1(0)
            transpose_pair(0)
            phase1(1)
            l1(0); l1(1)
            transpose_pair(1)
            l2l3(0); l2l3(1)
            l1(2); l1(3)
            l2l3(2); l2l3(3)

            # loss[p, c] -> dram [p, c]; host reorders to c*128+p
            nc.sync.dma_start(out_d[:, :], loss[:, :])

    nc.compile()
    return nc


_NC = None


def _get_nc():
    global _NC
    if _NC is None:
        _NC = build_nc()
    return _NC


def _input_maps(pred_corners, target_corners):
    cpack, ident = _build_constants()
    pred = np.ascontiguousarray(pred_corners, dtype=np.float32)
    targ = np.ascontiguousarray(target_corners, dtype=np.float32)
    in_maps = []
    for k in range(N_CORES):
        sl = slice(k * B_CORE, (k + 1) * B_CORE)
        # [4 chunks, 128 slots, 24] -> [128, 4*48] with per-chunk blocks
        # [pred_c | -targ_c]
        pk = pred[sl].reshape(N_CHUNKS, CHUNK, 24)
        tk = targ[sl].reshape(N_CHUNKS, CHUNK, 24)
        datak = np.concatenate([pk, -tk], axis=2)          # [4, 128, 48]
        datak = np.ascontiguousarray(
            datak.transpose(1, 0, 2).reshape(CHUNK, 192))
        in_maps.append({"data": datak, "cpack": cpack, "ident": ident})
    return in_maps


def _gather(results):
    outs = []
    for k in range(N_CORES):
        o = results[k]["out"].reshape(CHUNK, N_CHUNKS)
        outs.append(np.ascontiguousarray(o.T).reshape(B_CORE))
    return np.concatenate(outs)


def kernel(pred_corners: np.ndarray, target_corners: np.ndarray) -> np.ndarray:
    from concourse.bass_utils import run_bass_kernel_spmd

    nc = _get_nc()
    in_maps = _input_maps(pred_corners, target_corners)
    res = run_bass_kernel_spmd(nc, in_maps, core_ids=list(range(N_CORES)))
    return _gather(res.results)


# revision 16
# speedup vs baseline: 1.1811x; 1.1396x over previous
"""Trainium2 Bass kernel for CornerBoundingBoxEMDLoss.

For each sample: 8x8 pairwise corner distances, then exact min-cost perfect
matching via meet-in-the-middle:

  min over perms = min over 70 4-subsets T of
      (min assignment of preds {0,1,2,3} onto T)
    + (min assignment of preds {4,5,6,7} onto complement(T))

computed hierarchically: pred pairs -> target pairs (L1, one-hot GEMM with
two orderings + elementwise min), pairs -> quads (L2, one-hot GEMM over the
6 = C(4,2) pair-to-half assignments per 2+2 split + group-min), then a fused
add+min reduction over the 70 complement-aligned A+B sums (L3). Exact same
minimum as brute force over 8! permutations, ~50x less arithmetic.

Data-parallel across 8 NeuronCores: 512 samples per core, processed as
4 chunks of 128 samples (samples on SBUF partitions; d2 rows of two chunks
are transposed together in one PE pass to coord-major for the selection
GEMMs). Selection GEMMs run in bf16 (one-hot weights are exact in bf16;
distances round to ~0.4% which is well inside the 2e-2 gate). All inputs
arrive in 3 packed DMAs; the output leaves as one [128,4] DMA that the host
reorders.
"""

import itertools

import numpy as np
import ml_dtypes

import concourse.bacc as bacc
import concourse.mybir as mybir
import concourse.tile as tile

N_CORES = 8
B_TOTAL = 4096
B_CORE = B_TOTAL // N_CORES          # 512
N_CHUNKS = 4
CHUNK = B_CORE // N_CHUNKS           # 128

F32 = mybir.dt.float32
BF16 = mybir.dt.bfloat16

MIN_INIT = 1.0e30


def _build_constants():
    """Packed one-hot selection matrices + identity.

    cpack [128, 1064] bf16:
      cols   0:112  l1 ordering 0   (partitions 0:64 and replicated 64:128)
      cols 112:224  l1 ordering 1   (same replication)
      cols 224:1064 l2 (partitions 0:112): 840 = [A-side 70*6 | B-side 70*6]
    ident [128, 128] f32 for PE transposes.
    """
    pairs = list(itertools.combinations(range(8), 2))            # 28
    pair_idx = {p: i for i, p in enumerate(pairs)}
    subs4 = list(itertools.combinations(range(8), 4))            # 70
    pred_pairs = [(0, 1), (2, 3), (4, 5), (6, 7)]

    l1o0 = np.zeros((64, 112), dtype=np.float32)
    l1o1 = np.zeros((64, 112), dtype=np.float32)
    for q, (i0, i1) in enumerate(pred_pairs):
        for p, (a, b) in enumerate(pairs):
            col = q * 28 + p
            l1o0[i0 * 8 + a, col] = 1; l1o0[i1 * 8 + b, col] = 1
            l1o1[i0 * 8 + b, col] = 1; l1o1[i1 * 8 + a, col] = 1

    # all 6 C(T,2) choices of which target pair the first pred pair gets
    # (each 2+2 split appears twice with the pair roles swapped -- those are
    # distinct matchings, both needed)
    l2 = np.zeros((112, 840), dtype=np.float32)
    for t, T in enumerate(subs4):
        for s, S in enumerate(itertools.combinations(T, 2)):
            R = tuple(sorted(set(T) - set(S)))
            l2[0 * 28 + pair_idx[S], t * 6 + s] = 1
            l2[1 * 28 + pair_idx[R], t * 6 + s] = 1
        TB = tuple(sorted(set(range(8)) - set(T)))               # complement
        for s, S in enumerate(itertools.combinations(TB, 2)):
            R = tuple(sorted(set(TB) - set(S)))
            l2[2 * 28 + pair_idx[S], 420 + t * 6 + s] = 1
            l2[3 * 28 + pair_idx[R], 420 + t * 6 + s] = 1

    cpack = np.zeros((128, 1064), dtype=np.float32)
    cpack[0:64, 0:112] = l1o0
    cpack[0:64, 112:224] = l1o1
    cpack[64:128, 0:224] = cpack[0:64, 0:224]
    cpack[0:112, 224:1064] = l2
    cpack = cpack.astype(ml_dtypes.bfloat16)

    ident = np.eye(128, dtype=np.float32)
    return cpack, ident


def build_nc():
    nc = bacc.Bacc("TRN2", target_bir_lowering=False, debug=False)

    # data: per chunk c the 48-col block [pred_c | -targ_c], see kernel()
    data_d = nc.dram_tensor("data", [CHUNK, 48 * N_CHUNKS], F32,
                            kind="ExternalInput")
    cpack_d = nc.dram_tensor("cpack", [128, 1064], BF16, kind="ExternalInput")
    id_d = nc.dram_tensor("ident", [128, 128], F32, kind="ExternalInput")
    out_d = nc.dram_tensor("out", [CHUNK, N_CHUNKS], F32, kind="ExternalOutput")

    with tile.TileContext(nc) as tc:
        with (
            tc.tile_pool(name="consts", bufs=1) as cpool,
            tc.tile_pool(name="persist", bufs=1) as ppool,
            tc.tile_pool(name="work", bufs=3) as wpool,
            tc.tile_pool(name="pairs", bufs=2) as qpool,
            tc.tile_pool(name="psum_t", bufs=1, space="PSUM") as pst,
            tc.tile_pool(name="psum_a", bufs=2, space="PSUM") as psa,
            tc.tile_pool(name="psum_l2", bufs=2, space="PSUM") as psl2,
        ):
            data = cpool.tile([CHUNK, 192], F32, tag="data")
            cpk = cpool.tile([128, 1064], BF16, tag="cpack")
            c_id = cpool.tile([128, 128], F32, tag="ident")
            # data first (gates the whole pipeline), on the sync queue;
            # consts on the scalar queue in parallel; ident second on sync.
            nc.sync.dma_start(data[:, :], data_d[:, :])
            nc.scalar.dma_start(cpk[:, :], cpack_d[:, :])
            nc.sync.dma_start(c_id[:, :], id_d[:, :])

            m_t = ppool.tile([112, B_CORE], BF16, tag="m")
            loss = ppool.tile([128, N_CHUNKS], F32, tag="loss")
            tiny = ppool.tile([1, 1], F32, tag="tiny")

            # Force the (single) act table that holds sqrt+square+copy: the
            # table-load pass picks the table of the first activation, and
            # sqrt_and_others covers everything we use.
            nc.gpsimd.memset(tiny[:, :], 1.0)
            nc.scalar.activation(tiny[:, :], tiny[:, :],
                                 mybir.ActivationFunctionType.Sqrt)

            d2p = [None, None]
            dtp = [None, None]

            def phase1(c):
                """distances^2 for chunk c -> d2 pair tile column half."""
                pair, half = divmod(c, 2)
                if half == 0:
                    d2p[pair] = qpool.tile([CHUNK, 128], F32, tag="d2p", name="d2p")
                dsl = data[:, 48 * c: 48 * c + 48]
                diff = wpool.tile([CHUNK, 192], F32, tag="diff")
                p_b = (dsl[:, 0:24].rearrange("p (i c) -> p i c", i=8)
                       .unsqueeze(2).broadcast_to((CHUNK, 8, 8, 3)))
                t_b = (dsl[:, 24:48].rearrange("p (j c) -> p j c", j=8)
                       .unsqueeze(1).broadcast_to((CHUNK, 8, 8, 3)))
                d4 = diff[:, :].rearrange("p (i j c) -> p i j c", i=8, j=8)
                nc.gpsimd.tensor_add(d4, p_b, t_b)

                sq = wpool.tile([CHUNK, 192], BF16, tag="sq")
                nc.scalar.activation(sq[:, :], diff[:, :],
                                     mybir.ActivationFunctionType.Square)
                nc.vector.tensor_reduce(
                    d2p[pair][:, 64 * half: 64 * half + 64],
                    sq[:, :].rearrange("p (r c) -> p r c", c=3),
                    axis=mybir.AxisListType.X, op=mybir.AluOpType.add)

            def transpose_pair(pair):
                """[128 samples, 2x64 d2] -> bf16 dist [2x64, 128 samples]."""
                tp = pst.tile([128, 128], F32, tag="tp")
                nc.tensor.transpose(tp[:, :], d2p[pair][:, :], c_id[:, :])
                dtp[pair] = qpool.tile([128, 128], BF16, tag="dtp", name="dtp")
                nc.scalar.activation(dtp[pair][:, :], tp[:, :],
                                     mybir.ActivationFunctionType.Sqrt)

            def l1(c):
                """pred-pair x target-pair costs for chunk c -> m_t cols."""
                pair, half = divmod(c, 2)
                hp = slice(64 * half, 64 * half + 64)
                rhs = dtp[pair][hp, :]
                ps01 = psa.tile([112, 256], F32, tag="ps01")
                nc.tensor.matmul(ps01[:, 0:128], cpk[hp, 0:112], rhs,
                                 start=True, stop=True)
                nc.tensor.matmul(ps01[:, 128:256], cpk[hp, 112:224], rhs,
                                 start=True, stop=True)
                # HW: TensorTensor may read at most one input from PSUM
                s1 = wpool.tile([112, 128], F32, tag="s1")
                nc.scalar.activation(s1[:, :], ps01[:, 128:256],
                                     mybir.ActivationFunctionType.Copy)
                nc.vector.tensor_tensor(
                    m_t[:, CHUNK * c: CHUNK * (c + 1)], ps01[:, 0:128],
                    s1[:, :], op=mybir.AluOpType.min)

            def l2_mm(c):
                """quad-cost GEMM for chunk c -> [A | gap | B] psum."""
                ps2 = psl2.tile([128, 1024], F32, tag="ps2")
                msl = m_t[:, CHUNK * c: CHUNK * (c + 1)]
                nc.tensor.matmul(ps2[:, 0:420], msl, cpk[0:112, 224:644],
                                 start=True, stop=True)
                nc.tensor.matmul(ps2[:, 512:932], msl, cpk[0:112, 644:1064],
                                 start=True, stop=True)
                return ps2

            def minred_direct(c, ps2, split=False):
                """min over the 6 assignments, read straight from psum."""
                minab = wpool.tile([128, 140], BF16, tag="minab")
                if split:
                    # two half reduces: A-side overlaps the B-side matmul
                    nc.vector.tensor_reduce(
                        minab[:, 0:70],
                        ps2[:, 0:420].rearrange("p (t s) -> p t s", s=6),
                        axis=mybir.AxisListType.X, op=mybir.AluOpType.min)
                    nc.vector.tensor_reduce(
                        minab[:, 70:140],
                        ps2[:, 512:932].rearrange("p (t s) -> p t s", s=6),
                        axis=mybir.AxisListType.X, op=mybir.AluOpType.min)
                else:
                    v = (ps2[:, :].rearrange("p (h x) -> p h x", h=2)
                         [:, :, 0:420].rearrange("p h (t s) -> p h t s", s=6))
                    nc.vector.tensor_reduce(
                        minab[:, :], v,
                        axis=mybir.AxisListType.X, op=mybir.AluOpType.min)
                return minab

            def minred_via_scalar(c, ps2):
                """scalar psum->bf16 copy, then 2x-rate bf16 min-reduce."""
                cps = wpool.tile([128, 840], BF16, tag="cps")
                nc.scalar.activation(
                    cps[:, :].rearrange("p (h x) -> p h x", h=2),
                    ps2[:, :].rearrange("p (h x) -> p h x", h=2)[:, :, 0:420],
                    mybir.ActivationFunctionType.Copy)
                minab = wpool.tile([128, 140], BF16, tag="minab")
                nc.vector.tensor_reduce(
                    minab[:, :],
                    cps[:, :].rearrange("p (h t s) -> p h t s", h=2, s=6),
                    axis=mybir.AxisListType.X, op=mybir.AluOpType.min)
                return minab

            def l3(c, minab):
                scratch = wpool.tile([128, 70], BF16, tag="scratch")
                nc.vector.tensor_tensor(scratch[:, :], minab[:, 0:70],
                                        minab[:, 70:140],
                                        op=mybir.AluOpType.add)
                nc.vector.tensor_reduce(loss[:, c:c + 1], scratch[:, :],
                                        axis=mybir.AxisListType.X,
                                        op=mybir.AluOpType.min)

            # pipelined schedule (engine streams stay dependency-ordered):
            phase1(0); phase1(1)
            transpose_pair(0)
            phase1(2); phase1(3)
            l1(0); l1(1)
            transpose_pair(1)
            l1(2); l1(3)
            ps2_0 = l2_mm(0)
            m0 = minred_direct(0, ps2_0)
            ps2_1 = l2_mm(1)
            m1 = minred_via_scalar(1, ps2_1)
            l3(0, m0)
            ps2_2 = l2_mm(2)
            m2 = minred_via_scalar(2, ps2_2)
            l3(1, m1)
            ps2_3 = l2_mm(3)
            m3 = minred_direct(3, ps2_3, split=True)
            l3(2, m2)
            l3(3, m3)

            # loss[p, c] -> dram [p, c]; host reorders to c*128+p
            nc.sync.dma_start(out_d[:, :], loss[:, :])

    nc.compile()
    return nc


_NC = None


def _get_nc():
    global _NC
    if _NC is None:
        _NC = build_nc()
    return _NC


def _input_maps(pred_corners, target_corners):
    cpack, ident = _build_constants()
    pred = np.ascontiguousarray(pred_corners, dtype=np.float32)
    targ = np.ascontiguousarray(target_corners, dtype=np.float32)
    in_maps = []
    for k in range(N_CORES):
        sl = slice(k * B_CORE, (k + 1) * B_CORE)
        # [4 chunks, 128 slots, 24] -> [128, 4*48] with per-chunk blocks
        # [pred_c | -targ_c]
        pk = pred[sl].reshape(N_CHUNKS, CHUNK, 24)
        tk = targ[sl].reshape(N_CHUNKS, CHUNK, 24)
        datak = np.concatenate([pk, -tk], axis=2)          # [4, 128, 48]
        datak = np.ascontiguousarray(
            datak.transpose(1, 0, 2).reshape(CHUNK, 192))
        in_maps.append({"data": datak, "cpack": cpack, "ident": ident})
    return in_maps


def _gather(results):
    outs = []
    for k in range(N_CORES):
        o = results[k]["out"].reshape(CHUNK, N_CHUNKS)
        outs.append(np.ascontiguousarray(o.T).reshape(B_CORE))
    return np.concatenate(outs)


def kernel(pred_corners: np.ndarray, target_corners: np.ndarray) -> np.ndarray:
    from concourse.bass_utils import run_bass_kernel_spmd

    nc = _get_nc()
    in_maps = _input_maps(pred_corners, target_corners)
    res = run_bass_kernel_spmd(nc, in_maps, core_ids=list(range(N_CORES)))
    return _gather(res.results)
